# revision 1
# baseline (speedup 1.0000x reference)
"""Trainium2 Bass kernel for the Alignment problem.

reference semantics (per batch):
    attn = (a @ b.T) * temperature                       # [LA, LB]
    mask = outer(mask_a, mask_b) > 0
    attn = where(mask, attn, -1e7)
    attn_a = softmax(attn, axis=0)   # over i (a-tokens)
    attn_b = softmax(attn, axis=1)   # over j (b-tokens)
    feature_b = attn_a.T @ a         # [LB, H]
    feature_a = attn_b @ b           # [LA, H]

Sharding: batch 16 -> 2 per core across 8 NeuronCores (pure data parallel).

Mask folding: with b' = temp*mask_b[j]*b[j,:] and one extra contraction row
10 x (10*mask_b[j]), the augmented matmul gives
    Praw[i,j] = temp*mask_b[j]*<a_i,b_j> + 100*mask_b[j].
The exp pass then applies a per-partition scale mask_a[i] (free on the
activation instruction), giving P = mask*(S+100): valid entries live in
[94+S, 106+S], masked entries are exactly 0.

Constant-shift softmax: softmax is shift invariant and |S| <= ~8 for the
reference setup (temp = 1/sqrt(H), randn inputs), so no row-max pass is
needed. E = exp(P - C) with runtime shift C (nominally 85): valid ->
e^(S+15) (no overflow), masked -> e^-85 = 1.2e-37 (normal, != 0). A fully
masked row is all-equal -> exactly uniform after normalization, matching
jax's softmax of a constant row. Masked entries in valid rows carry weight
~1e-40, matching the reference's exact 0 to f32 noise. For larger score
scales the host raises C (overflow safety) and patches fully-masked rows +
any residual nonfinite rows exactly on the host.

Because the shift is constant (not per-row), ONE matrix E serves BOTH
softmax directions; only the normalizers differ: Z_b[i] = row sums of E
(free ACT accumulator on the exp pass), Z_a[j] = row sums of E^T (free ACT
accumulator on the PSUM->SBUF copy after the PE transpose of E).
"""

import sys

sys.path.insert(0, "/opt/trn_rl_repo")

import numpy as np

import concourse.bass as bass
import concourse.tile as tile
from concourse import mybir
from concourse.masks import make_identity
from concourse.bass_utils import run_bass_kernel_spmd

B, LA, LB, H = 16, 1024, 1024, 512
NCORES = 8
BPC = B // NCORES  # batches per core
P = 128

FP32 = mybir.dt.float32
BF16 = mybir.dt.bfloat16
AF = mybir.ActivationFunctionType

NI = LA // P  # 8 i row-blocks
NJ = LB // P  # 8 j row-blocks
KC = H // P  # 4 contraction chunks of the H axis
NJH = LB // 512  # j halves (psum-bank-sized)


POOL_SPECS = [
    ("nat", 2, None),
    ("natbf", 1, None),
    ("tr", 2, None),
    ("esb", 1, None),
    ("etsb", 1, None),
    ("scr", 2, None),
    ("small", 2, None),
    ("stat", 4, None),
    ("rz", 1, None),
    ("outp", 3, None),
    ("ps_w", 2, "PSUM"),
    ("ps_score", 2, "PSUM"),
]


def emit_consts(nc, singles):
    ident32 = singles.tile([P, P], FP32, tag="ident32", name="ident32")
    make_identity(nc, ident32[:])
    identbf = singles.tile([P, P], BF16, tag="identbf", name="identbf")
    make_identity(nc, identbf[:])
    tens = singles.tile([1, P], BF16, tag="tens", name="tens")
    nc.vector.memset(tens[:], 10.0)
    return dict(ident32=ident32, identbf=identbf, tens=tens)


def emit_body(nc, pools, exts, consts):
    """Emit the full 2-batch per-core computation."""
    p_nat = pools["nat"]
    p_natbf = pools["natbf"]
    p_t = pools["tr"]
    p_e = pools["esb"]
    p_et = pools["etsb"]
    p_scr = pools["scr"]
    p_small = pools["small"]
    p_st = pools["stat"]
    p_rz = pools["rz"]
    p_out = pools["outp"]
    p_ps_w = pools["ps_w"]
    p_ps_s = pools["ps_score"]
    a_ext, b_ext = exts["a"], exts["b"]
    fas_ext, fbs_ext, fbr_ext = exts["fa_s"], exts["fb_s"], exts["fb_r"]
    oa_ext, ob_ext = exts["out_a"], exts["out_b"]
    identbf = consts["identbf"]
    tens = consts["tens"]

    # ---------------- load pass: both batches up front -----------
    # (keeps SP's DMA-trigger queue free of output-DMA waits, so
    # batch 2's inputs stream in during batch 1's compute).
    # Order within a batch follows the startup critical path: a row-blocks
    # first (convert -> transpose starts immediately), then fb_s + b
    # row-blocks (scale -> transpose), mask/shift scalars after.
    shiftc = None
    loads = []
    for bi in range(BPC):
        # batched loads: one DMA descriptor per 4 row-blocks (the DMA cost
        # is dominated by per-descriptor overhead, not bytes)
        An = [None] * NI
        Bn = [None] * NJ

        def load4(ext, names, rh, tag):
            t = p_nat.tile([P, 4, H], BF16, tag=f"{tag}{rh}", name=f"{tag}{rh}")
            nc.sync.dma_start(
                out=t[:],
                in_=ext[bi, rh * 512 : (rh + 1) * 512, :].rearrange(
                    "(r p) d -> p r d", p=P
                ),
            )
            for q in range(4):
                names[rh * 4 + q] = t[:, q, :]

        # aT chunks via transposing DMA straight from DRAM (bf16 supports
        # DMA transpose; the a-side needs no scaling, so no PE pass at all)
        aT = []
        for c in range(KC):
            t = p_t.tile([P, LA], BF16, tag=f"aT{c}", name=f"aT{c}")
            nc.sync.dma_start(
                out=t[:],
                in_=a_ext[bi, :, c * P : (c + 1) * P],
                transpose=True,
            )
            aT.append(t)
        fb_s = p_small.tile([P, NJ], FP32, tag=f"fb_s{bi}", name=f"fb_s{bi}")
        nc.sync.dma_start(
            out=fb_s[:], in_=fbs_ext[bi].rearrange("(h p) -> p h", p=P)
        )
        load4(b_ext, Bn, 0, "Bn4_")
        fb_r0 = p_small.tile([1, LB], FP32, tag=f"fb_r0{bi}", name=f"fb_r0{bi}")
        nc.sync.dma_start(out=fb_r0[:], in_=fbr_ext[bi : bi + 1, :])
        fa_s = p_small.tile([P, NI], FP32, tag=f"fa_s{bi}", name=f"fa_s{bi}")
        nc.sync.dma_start(
            out=fa_s[:], in_=fas_ext[bi].rearrange("(h p) -> p h", p=P)
        )
        if shiftc is None:
            # negative exp shift, broadcast from a runtime scalar input
            shiftc = p_small.tile([P, 1], FP32, tag="shiftc", name="shiftc")
            nc.sync.dma_start(
                out=shiftc[:], in_=exts["shift"][:].to_broadcast([P, 1])
            )
        load4(b_ext, Bn, 1, "Bn4_")
        load4(a_ext, An, 0, "An4_")
        load4(a_ext, An, 1, "An4_")
        loads.append((fa_s, fb_s, fb_r0, An, Bn, aT))

    for bi in range(BPC):
        fa_s, fb_s, fb_r0, An, Bn, aT = loads[bi]
        # bf16 copy of the b mask row (0/10 exact in bf16)
        fb_r = p_small.tile([1, LB], BF16, tag="fb_r", name="fb_r")
        nc.vector.tensor_copy(out=fb_r[:], in_=fb_r0[:])

        # -------- phase 1: scale b rows by temp*mask, transpose -----
        bT = [
            p_t.tile([P, LB], BF16, tag=f"bT{c}", name=f"bT{c}")
            for c in range(KC)
        ]
        # a-side needs NO mask/temp scaling at all: mask_a rides the exp's
        # per-partition scale. The b-side temp*mask_b scale is folded into
        # the transpose itself: transpose(out, in_, rhs) = in_.T @ rhs, and
        # rhs = diag(temp*mask_b) scales column j by temp*mask_b[j]. Inputs
        # arrive pre-converted to bf16, so the loaded tiles serve directly
        # as the feature rhs AND the transpose sources.
        Anb, Bnb = An, Bn
        diagb = []
        for r in range(NJ):
            t = p_scr.tile([P, P], BF16, tag=f"diagb{r}", name=f"diagb{r}")
            nc.vector.tensor_scalar_mul(t[:], identbf[:], fb_s[:, r : r + 1])
            diagb.append(t)
        # one 512-wide copy per (chunk, half) so every matmul operand
        # slice has a single producer (sync-wait limit per matmul).
        for rh in range(2):
            for src_, dst, diags in ((Bnb, bT, diagb),):
                for c in range(KC):
                    ptdt = BF16 if diags is None else FP32
                    pttag = "w512tp" if diags is None else "w512"
                    pt = p_ps_w.tile([P, H], ptdt, tag=pttag, name="w512tp")
                    for q in range(4):
                        r = rh * 4 + q
                        if diags is None:
                            nc.tensor.transpose(
                                pt[:, q * P : (q + 1) * P],
                                src_[r][:, c * P : (c + 1) * P],
                                identbf[:],
                            )
                        else:
                            # scaled transpose via the REAL matmul path
                            # (transpose-mode is a permutation that ignores
                            # rhs values): out = b_chunk.T @ diag(temp*mask)
                            nc.tensor.matmul(
                                pt[:, q * P : (q + 1) * P],
                                lhsT=src_[r][:, c * P : (c + 1) * P],
                                rhs=diags[r][:],
                                start=True,
                                stop=True,
                            )
                    nc.vector.tensor_copy(
                        out=dst[c][:, rh * 512 : (rh + 1) * 512], in_=pt[:]
                    )

        # ---- phase 2: score matmul + shared exp (one direction) ----
        E, rzb = [], []
        for ib in range(NI):
            s2 = p_ps_s.tile([P, LB], FP32, tag="score", name="score")
            for jh in range(NJH):
                seg = s2[:, jh * 512 : (jh + 1) * 512]
                for c in range(KC):
                    nc.tensor.matmul(
                        seg,
                        lhsT=aT[c][:, ib * P : (ib + 1) * P],
                        rhs=bT[c][:, jh * 512 : (jh + 1) * 512],
                        start=(c == 0),
                        stop=False,
                    )
                nc.tensor.matmul(
                    seg,
                    lhsT=tens[0:1, :],
                    rhs=fb_r[0:1, jh * 512 : (jh + 1) * 512],
                    start=False,
                    stop=True,
                )
            e = p_e.tile([P, LB], BF16, tag=f"E{ib}", name=f"E{ib}")
            z = p_st.tile([P, 1], FP32, tag="z", name="z")
            nc.scalar.activation(
                out=e[:],
                in_=s2[:],
                func=AF.Exp,
                bias=shiftc[:],
                scale=fa_s[:, ib : ib + 1],
                accum_out=z[:],
            )
            rz = p_rz.tile([P, 1], FP32, tag=f"rzb{ib}", name=f"rzb{ib}")
            nc.vector.reciprocal(rz[:], z[:])
            E.append(e)
            rzb.append(rz)

        # ---- phase 4: transpose E; Z_a free via ACT copy accum -----
        ET, rza = [], []
        for jb in range(NJ):
            et = p_et.tile([P, LA], BF16, tag=f"ET{jb}", name=f"ET{jb}")
            zparts = []
            for hh in range(2):
                etp = p_ps_w.tile([P, 512], BF16, tag="w512tp", name="etp")
                for q in range(4):
                    ib = hh * 4 + q
                    nc.tensor.transpose(
                        etp[:, q * P : (q + 1) * P],
                        E[ib][:, jb * P : (jb + 1) * P],
                        identbf[:],
                    )
                zp = p_st.tile([P, 1], FP32, tag="zp", name="zp")
                nc.scalar.activation(
                    out=et[:, hh * 512 : (hh + 1) * 512],
                    in_=etp[:],
                    func=AF.Copy,
                    bias=0.0,
                    scale=1.0,
                    accum_out=zp[:],
                )
                zparts.append(zp)
            za = p_st.tile([P, 1], FP32, tag="za", name="za")
            nc.vector.tensor_add(za[:], zparts[0][:], zparts[1][:])
            rz = p_rz.tile([P, 1], FP32, tag=f"rza{jb}", name=f"rza{jb}")
            nc.vector.reciprocal(rz[:], za[:])
            ET.append(et)
            rza.append(rz)

        # ------------- phase 5: feature matmuls + normalize ---------
        for jb in range(NJ):
            f = p_ps_w.tile([P, H], FP32, tag="w512", name="w512")
            for ic in range(NI):
                nc.tensor.matmul(
                    f[:],
                    lhsT=E[ic][:, jb * P : (jb + 1) * P],
                    rhs=Anb[ic][:],
                    start=(ic == 0),
                    stop=(ic == NI - 1),
                )
            ob = p_out.tile([P, H], FP32, tag="ob", name="ob")
            nc.vector.tensor_scalar_mul(ob[:], f[:], rza[jb][:])
            nc.sync.dma_start(out=ob_ext[bi, jb * P : (jb + 1) * P, :], in_=ob[:])
        for ib in range(NI):
            f = p_ps_w.tile([P, H], FP32, tag="w512", name="w512")
            for jc in range(NJ):
                nc.tensor.matmul(
                    f[:],
                    lhsT=ET[jc][:, ib * P : (ib + 1) * P],
                    rhs=Bnb[jc][:],
                    start=(jc == 0),
                    stop=(jc == NJ - 1),
                )
            oa = p_out.tile([P, H], FP32, tag="oa", name="oa")
            nc.vector.tensor_scalar_mul(oa[:], f[:], rzb[ib][:])
            nc.sync.dma_start(out=oa_ext[bi, ib * P : (ib + 1) * P, :], in_=oa[:])


def declare_exts(nc):
    return dict(
        a=nc.declare_dram_parameter("a", [BPC, LA, H], BF16, isOutput=False),
        b=nc.declare_dram_parameter("b", [BPC, LB, H], BF16, isOutput=False),
        fa_s=nc.declare_dram_parameter("fa_s", [BPC, LA], FP32, isOutput=False),
        fb_s=nc.declare_dram_parameter("fb_s", [BPC, LB], FP32, isOutput=False),
        fb_r=nc.declare_dram_parameter("fb_r", [BPC, LB], FP32, isOutput=False),
        shift=nc.declare_dram_parameter("shift", [1, 1], FP32, isOutput=False),
        out_a=nc.declare_dram_parameter("out_a", [BPC, LA, H], FP32, isOutput=True),
        out_b=nc.declare_dram_parameter("out_b", [BPC, LB, H], FP32, isOutput=True),
    )


def build_nc() -> bass.Bass:
    import contextlib

    nc = bass.Bass()
    exts = declare_exts(nc)
    with tile.TileContext(nc) as tc, contextlib.ExitStack() as ctx:
        singles = ctx.enter_context(tc.tile_pool(name="singles", bufs=1))
        pools = {
            name: ctx.enter_context(
                tc.tile_pool(name=name, bufs=bufs, space=space)
                if space
                else tc.tile_pool(name=name, bufs=bufs)
            )
            for name, bufs, space in POOL_SPECS
        }
        consts = emit_consts(nc, singles)
        emit_body(nc, pools, exts, consts)
    return nc


def legalize_waits(nc: bass.Bass, cap_default: int = 1, cap_evsem: int = 2):
    """Walrus in this toolchain accepts only one embedded sync-wait per TPB
    instruction. Hoist excess waits onto standalone InstEventSemaphore
    instructions (<=2 waits each) on the same engine, preceding the
    instruction, which preserves per-engine program-order semantics."""
    for f in nc.m.functions:
        for blk in f.blocks:
            new = []
            for inst in blk.instructions:
                si = inst.sync_info
                if (
                    si is not None
                    and si.on_wait
                    and not isinstance(inst, mybir.InstEventSemaphore)
                    and len(si.on_wait) > cap_default
                ):
                    waits = list(si.on_wait)
                    keep, extra = waits[:cap_default], waits[cap_default:]
                    while extra:
                        chunk, extra = extra[:cap_evsem], extra[cap_evsem:]
                        new.append(
                            mybir.InstEventSemaphore(
                                name=nc.get_next_instruction_name(),
                                engine=inst.engine,
                                ins=[],
                                outs=[],
                                sync_info=mybir.SyncInfo(on_wait=chunk, on_update=[]),
                            )
                        )
                    si.on_wait = keep
                new.append(inst)
            blk.instructions[:] = new


_NC = None
LAST = None  # BassKernelResults of the most recent run (for test harness)


def kernel(a, b, mask_a, mask_b, temperature):
    global _NC, LAST
    import ml_dtypes

    a = np.ascontiguousarray(np.asarray(a, dtype=np.float32))
    b = np.ascontiguousarray(np.asarray(b, dtype=np.float32))
    a16 = np.ascontiguousarray(a.astype(ml_dtypes.bfloat16))
    b16 = np.ascontiguousarray(b.astype(ml_dtypes.bfloat16))
    ma = np.asarray(mask_a).astype(np.float32).reshape(B, LA)
    mb = np.asarray(mask_b).astype(np.float32).reshape(B, LB)
    temp = float(np.asarray(temperature))

    if _NC is None:
        _NC = build_nc()
        legalize_waits(_NC)

    # Safe constant exp shift. Nominal (reference setup: temp = 1/sqrt(H),
    # randn inputs -> scores ~N(0,1)) uses 85, which also keeps fully-masked
    # rows exact on device (exp(-85) is normal, nonzero). For larger score
    # scales, raise the shift to avoid exp overflow; fully-masked rows then
    # hit Z=0 on device and are patched on the host below (their reference
    # value is just the mean of the other operand's rows).
    sigma = temp * float(np.sqrt(H * max(a.var(), 1e-30) * max(b.var(), 1e-30)))
    shift_val = max(85.0, 15.0 + 6.5 * sigma)

    in_maps = []
    for c in range(NCORES):
        sl = slice(c * BPC, (c + 1) * BPC)
        in_maps.append(
            {
                "a": a16[sl],
                "b": b16[sl],
                "fa_s": np.ascontiguousarray(ma[sl]),
                "fb_s": np.ascontiguousarray(mb[sl] * temp),
                "fb_r": np.ascontiguousarray(mb[sl] * 10.0),
                "shift": np.full((1, 1), -shift_val, np.float32),
            }
        )

    LAST = run_bass_kernel_spmd(_NC, in_maps, core_ids=list(range(NCORES)))
    feature_a = np.concatenate([r["out_a"] for r in LAST.results], axis=0)
    feature_b = np.concatenate([r["out_b"] for r in LAST.results], axis=0)
    if shift_val > 87.0:
        # off-nominal score scale: fully-masked rows underflowed to Z=0 on
        # device (their reference value is simply the mean of the other
        # operand's rows)
        for bi in range(B):
            feature_a[bi, ma[bi] == 0.0, :] = b[bi].mean(axis=0)
            feature_b[bi, mb[bi] == 0.0, :] = a[bi].mean(axis=0)

    # safety net: exactly recompute any residual nonfinite rows (e.g. a
    # single >6.5-sigma score overflowing the constant-shift exp). Nominal
    # inputs never trigger this; the check itself is a cheap scan.
    def _fix_rows(feat, this, other, row_mask, col_mask):
        bad_b, bad_r = np.nonzero(~np.isfinite(feat).all(axis=2))
        for bi, r in zip(bad_b, bad_r):
            srow = (other[bi] @ this[bi, r]) * temp  # scores vs. all others
            srow = np.where(
                (row_mask[bi, r] * col_mask[bi]) > 0, srow, -1e7
            ).astype(np.float64)
            srow -= srow.max()
            w = np.exp(srow)
            w /= w.sum()
            feat[bi, r, :] = (w @ other[bi]).astype(np.float32)

    if not np.isfinite(feature_a).all() or not np.isfinite(feature_b).all():
        _fix_rows(feature_a, a, b, ma, mb)
        _fix_rows(feature_b, b, a, mb, ma)
    return feature_a, feature_b

